# revision 15
# baseline (speedup 1.0000x reference)
"""Trainium2 Bass kernel for nn_MultiHeadAttention (B=2, N=M=2048, D=1024, H=16).

Sharding: 8 cores = 2 batches x 4 head-groups (4 heads per core, tensor-parallel
over the head dim of Wq/Wk/Wv/Wp).  Each core computes a partial output
projection [N, D]; the host sums the 4 partials per batch and adds bp.

v3 dataflow (ACT-saturating pipeline; ACT's exp of the 4 x 2048 x 2048 score
matrix at 1 elem/lane/cycle is the hard floor, ~128us/core):
  - attention runs over 4 n-strips of 512.  Per (strip, g, m-chunk): two
    64-contraction score matmuls (heads 2g / 2g+1, PE row-tiles at partition
    bases 0/64 run them concurrently) into a double-buffered [128, 2, 512]
    PSUM pair; ONE ACT exp op over the full [128, 1024] with the 1/sqrt(dh)
    scale folded in; ONE DVE f16 mask multiply against a host-duplicated
    [m, 2, n] mask strip; two AV matmuls accumulating [65, 512] per head
    (ones-column = softmax denominator).
  - everything else rides in the PE/DVE slack under ACT via a thunk queue
    drained one item per m-iteration: the K/V/Q projections (prelude work
    interleaves into strip 0 so the first exp fires ~10us in; an up-front
    dummy exp preloads the ACT table during the DMA ramp), the Q^T
    projection for strip s+1, and strip s's tail (denominator gather,
    PE-transpose -> f16 reciprocal -> broadcast matmul -> in-place
    normalize, then the output projection as 4 x K=64 matmuls per [128,512]
    tile).  The queue carries across reps, so rep r's last tail overlaps
    rep r+1's prelude.
"""

import os
import numpy as np
from contextlib import ExitStack

import concourse.bass as bass
import concourse.tile as tile
from concourse import mybir
from concourse.bass_utils import run_bass_kernel_spmd
from concourse.vector_clock import ScopedClock
from concourse.masks import make_identity

B, N, M, D = 2, 2048, 2048, 1024
H = 16
DH = D // H  # 64
SCALE = DH ** -0.5
NCORES = 8
HG = 4            # heads per core
CSL = HG * DH     # 256 columns of Wq/Wk/Wv per core
F32 = mybir.dt.float32
F16 = mybir.dt.float16

NSTRIP = 512          # attention n-strip width
NS = N // NSTRIP      # 4 strips
MT = M // 128         # 16 m-chunks

_ABLATE = os.environ.get("ABLATE", "")  # comma-list: nomask, noexp

# ---------------------------------------------------------------------------
# walrus in this container rejects >1 sem wait per instruction; spread the
# extras across preceding same-engine NOPs (queues execute in order, so this
# is semantically identical).
_MAX_WAITS = 1


def _patched_drain_and_barrier(self, tick_clock, wait_clock):
    drain_inst = self.nc.sync.drain()
    wait_clock.add_sem_waits(
        drain_inst.ins, ScopedClock({None: tick_clock.global_clock})
    )
    si = drain_inst.ins.sync_info
    waits = list(si.on_wait or []) if si else []
    if len(waits) > _MAX_WAITS:
        si.on_wait = waits[:_MAX_WAITS]
        for i in range(_MAX_WAITS, len(waits), _MAX_WAITS):
            extra = self.nc.sync.drain()
            extra.ins.sync_info = mybir.SyncInfo(
                on_wait=waits[i : i + _MAX_WAITS], on_update=[]
            )
    self.nc.all_engine_barrier()
    assert self.sems is not None
    popped = self.nc._tile_sem_poison_stack.pop()
    assert popped is self._sem_poison
    self.nc.clear_and_free_semaphores(list(self.sems.allocated().values()))
    self.nc.all_engine_barrier()


tile.TileContext._drain_and_barrier = _patched_drain_and_barrier
# ---------------------------------------------------------------------------

Exp = mybir.ActivationFunctionType.Exp
Identity = mybir.ActivationFunctionType.Identity


def _split_waits(nc):
    n_split = 0
    for bb in nc.main_func.blocks:
        new_list = []
        for ins in bb.instructions:
            si = ins.sync_info
            if si is not None and si.on_wait and len(si.on_wait) > 1:
                waits = list(si.on_wait)
                for j, w in enumerate(waits[:-1]):
                    nop = mybir.InstNoOp(
                        name=f"{ins.name}-sw{j}",
                        engine=ins.engine,
                        sync_info=mybir.SyncInfo(on_wait=[w], on_update=[]),
                    )
                    new_list.append(nop)
                    n_split += 1
                si.on_wait = [waits[-1]]
            new_list.append(ins)
        bb.instructions = new_list
    return n_split


def build_nc(reps: int = 1) -> bass.Bass:
    nc = bass.Bass()

    xqT = nc.dram_tensor("xqT", [D, N], F16, kind="ExternalInput")
    xkvT = nc.dram_tensor("xkvT", [D, M], F16, kind="ExternalInput")
    maskdT = nc.dram_tensor("maskdT", [M, 2, N], F16, kind="ExternalInput")
    wq = nc.dram_tensor("wq", [D, CSL], F16, kind="ExternalInput")
    wk = nc.dram_tensor("wk", [D, CSL], F16, kind="ExternalInput")
    wv = nc.dram_tensor("wv", [D, CSL], F16, kind="ExternalInput")
    wp = nc.dram_tensor("wp", [CSL, D], F16, kind="ExternalInput")
    bq2 = nc.dram_tensor("bq2", [128, 2], F32, kind="ExternalInput")
    bk2 = nc.dram_tensor("bk2", [128, 2], F32, kind="ExternalInput")
    bv1 = nc.dram_tensor("bv1", [1, CSL], F16, kind="ExternalInput")
    sel4in = nc.dram_tensor("sel4in", [4, HG * DH], F16, kind="ExternalInput")
    outp = nc.dram_tensor("outp", [N, D], F16, kind="ExternalOutput")

    with ExitStack() as ctx:
        tc = ctx.enter_context(tile.TileContext(nc))

        consts = ctx.enter_context(tc.tile_pool(name="consts", bufs=1))
        ident = consts.tile([128, 128], F16)
        make_identity(nc, ident)
        # 6000*I: folds the mask into score PSUM (out += 6000*mask), paired
        # with bias=-750 inside the exp so masked entries underflow to 0.
        ident6k = consts.tile([128, 128], F16)
        make_identity(nc, ident6k)
        nc.vector.tensor_scalar_mul(ident6k, ident6k, 6000.0)
        ones_row = consts.tile([1, 128], F16)
        nc.vector.memset(ones_row, 1.0)
        sel4 = consts.tile([4, 4, DH], F16)
        nc.sync.dma_start(out=sel4, in_=sel4in[:, :])
        bq_sb = consts.tile([128, 2], F32)
        nc.sync.dma_start(out=bq_sb, in_=bq2[:, :])
        bk_sb = consts.tile([128, 2], F32)
        nc.sync.dma_start(out=bk_sb, in_=bk2[:, :])
        bv_sb = consts.tile([1, CSL], F16)
        nc.sync.dma_start(out=bv_sb, in_=bv1[:, :])
        # wp per head at partitions 0-63: [64, h, slice, 512]
        wp_sb = consts.tile([DH, HG, 2, 512], F16)
        for h in range(HG):
            nc.sync.dma_start(
                out=wp_sb[:, h, :, :], in_=wp[h * DH : (h + 1) * DH, :]
            )
        neg750 = consts.tile([128, 1], F32)
        nc.vector.memset(neg750, -750.0)
        # preload the exp table set while the first DMAs run
        warm = consts.tile([1, 2], F32)
        nc.vector.memset(warm, 0.0)
        nc.scalar.activation(warm, warm, Exp, scale=1.0)

        persist = ctx.enter_context(tc.tile_pool(name="persist", bufs=1))
        KT = [persist.tile([128, M], F16, tag=f"kt{g}", name=f"kt{g}") for g in range(2)]
        QT = [persist.tile([128, N], F16, tag=f"qt{g}", name=f"qt{g}") for g in range(2)]
        V = persist.tile([128, MT, HG, DH + 1], F16, tag="v")
        wq_sb = persist.tile([128, 8, CSL], F16, tag="wq")
        for d in range(8):
            nc.sync.dma_start(out=wq_sb[:, d, :], in_=wq[d * 128 : (d + 1) * 128, :])

        maskp = ctx.enter_context(tc.tile_pool(name="maskp", bufs=2))
        xqp = ctx.enter_context(tc.tile_pool(name="xqp", bufs=2))
        xkvp = ctx.enter_context(tc.tile_pool(name="xkvp", bufs=2))
        wkvp = ctx.enter_context(tc.tile_pool(name="wkvp", bufs=1))
        etap = ctx.enter_context(tc.tile_pool(name="etap", bufs=4))
        otsp = ctx.enter_context(tc.tile_pool(name="otsp", bufs=2))
        obp = ctx.enter_context(tc.tile_pool(name="obp", bufs=2))
        nrmp = ctx.enter_context(tc.tile_pool(name="nrmp", bufs=2))

        stp = ctx.enter_context(tc.tile_pool(name="stp", bufs=2, space="PSUM"))
        otp = ctx.enter_context(tc.tile_pool(name="otp", bufs=1, space="PSUM"))
        tailp = ctx.enter_context(tc.tile_pool(name="tailp", bufs=2, space="PSUM"))

        pending = []  # emission thunks drained one per m-iteration

        def dma_xq_strip(s):
            xq_sb = xqp.tile([128, 8, NSTRIP], F16, tag="xq", name=f"xq{s}")
            for d in range(8):
                nc.sync.dma_start(
                    out=xq_sb[:, d, :],
                    in_=xqT[d * 128 : (d + 1) * 128, s * NSTRIP : (s + 1) * NSTRIP],
                )
            return xq_sb

        def dma_mask_strip(s):
            mkd = maskp.tile([128, MT, 2, 512], F16, tag="mk", name=f"mk{s}")
            for m in range(MT):
                nc.sync.dma_start(
                    out=mkd[:, m, :, :],
                    in_=maskdT[
                        m * 128 : (m + 1) * 128, :, s * NSTRIP : (s + 1) * NSTRIP
                    ],
                )
            return mkd

        def project_q(xq_sb, s, on_act=False):
            for g in range(2):
                ps = tailp.tile([128, NSTRIP], F32, tag="t", name=f"qps{s}{g}")
                for d in range(8):
                    nc.tensor.matmul(
                        ps,
                        wq_sb[:, d, g * 128 : (g + 1) * 128],
                        xq_sb[:, d, :],
                        start=(d == 0),
                        stop=(d == 7),
                    )
                dst = QT[g][:, s * NSTRIP : (s + 1) * NSTRIP]
                if on_act:
                    nc.scalar.activation(dst, ps, Identity, bias=bq_sb[:, g : g + 1])
                else:
                    nc.vector.tensor_scalar_add(dst, ps, bq_sb[:, g : g + 1])

        def project_k_tile(wk_sb, xkv_sb, g, mh, ms, on_act=False):
            ps = tailp.tile([128, 512], F32, tag="t", name="kps")
            for d in range(8):
                nc.tensor.matmul(
                    ps,
                    wk_sb[:, d, g * 128 : (g + 1) * 128],
                    xkv_sb[:, d, ms * 512 : (ms + 1) * 512],
                    start=(d == 0),
                    stop=(d == 7),
                )
            dst = KT[g][:, mh * 1024 + ms * 512 : mh * 1024 + (ms + 1) * 512]
            if on_act:
                nc.scalar.activation(dst, ps, Identity, bias=bk_sb[:, g : g + 1])
            else:
                nc.vector.tensor_scalar_add(dst, ps, bk_sb[:, g : g + 1])

        def project_v_pair(wv_sb, xkv_sb, mh, mt0):
            for mt in (mt0, mt0 + 1):
                vp = tailp.tile([128, HG, DH], F32, tag="t", name="vp")
                for d in range(8):
                    nc.tensor.matmul(
                        vp,
                        xkv_sb[:, d, mt * 128 : (mt + 1) * 128],
                        wv_sb[:, d, :],
                        start=(d == 0),
                        stop=False,
                    )
                nc.tensor.matmul(vp, ones_row, bv_sb, start=False, stop=True)
                nc.vector.tensor_copy(V[:, mh * 8 + mt, :, 0:DH], vp)

        def emit_tail(s, ots):
            """Normalize + output projection for strip s, as thunks."""

            def norm_chain(s=s, ots=ots):
                rowsums = nrmp.tile([4, NSTRIP], F16, tag="rs", name="rs")
                nc.sync.dma_start(out=rowsums, in_=ots[DH : DH + 1, :, :])
                rsT = tailp.tile([128, 16], F16, tag="t", name="rsT")
                for b in range(4):
                    nc.tensor.transpose(
                        rsT[:, 4 * b : 4 * b + 4],
                        rowsums[:, b * 128 : (b + 1) * 128],
                        ident[0:4, 0:4],
                    )
                recipT = nrmp.tile([128, 16], F16, tag="rcp", name="recipT")
                with nc.allow_low_precision(
                    reason="softmax denominators are O(1e3); f16 recip gives "
                    "~5e-4 rel err, well inside the output tolerance"
                ):
                    nc.vector.reciprocal(recipT, rsT)
                rrps = tailp.tile([4, NSTRIP], F16, tag="t", name="rrps")
                for b in range(4):
                    nc.tensor.transpose(
                        rrps[:, b * 128 : (b + 1) * 128],
                        recipT[:, 4 * b : 4 * b + 4],
                        ident,
                    )
                recrows = nrmp.tile([4, NSTRIP], F16, tag="rr", name="recrows")
                nc.vector.tensor_copy(recrows, rrps)
                emit_tail.recrows = recrows

            def norm_heads(h0, s=s, ots=ots):
                recrows = emit_tail.recrows
                for h in (h0, h0 + 1):
                    rps = tailp.tile([DH, NSTRIP], F32, tag="t", name="rps")
                    nc.tensor.matmul(rps, sel4[:, h, :], recrows)
                    nc.vector.tensor_mul(ots[0:DH, h, :], ots[0:DH, h, :], rps)

            def out_tile(t, s=s, ots=ots):
                ob = obp.tile([128, 2, 512], F16, tag="ob", name="ob")
                for sl in range(2):
                    po = tailp.tile([128, 512], F32, tag="t", name="po")
                    for h in range(HG):
                        nc.tensor.matmul(
                            po,
                            ots[0:DH, h, t * 128 : (t + 1) * 128],
                            wp_sb[:, h, sl, :],
                            start=(h == 0),
                            stop=(h == HG - 1),
                        )
                    nc.vector.tensor_copy(ob[:, sl, :], po)
                row = s * NSTRIP + t * 128
                nc.sync.dma_start(out=outp[row : row + 128, :], in_=ob)

            pending.append(norm_chain)
            pending.append(lambda: norm_heads(0))
            pending.append(lambda: norm_heads(2))
            for t in range(NSTRIP // 128):
                pending.append(lambda t=t: out_tile(t))

        for _rep in range(reps):
            # ---- prelude: DMAs + enough projections to start strip 0 ----
            wk_sb = wkvp.tile([128, 8, CSL], F16, tag="wk", name="wk_sb")
            wv_sb = wkvp.tile([128, 8, CSL], F16, tag="wv", name="wv_sb")
            for d in range(8):
                nc.sync.dma_start(out=wk_sb[:, d, :], in_=wk[d * 128 : (d + 1) * 128, :])
                nc.sync.dma_start(out=wv_sb[:, d, :], in_=wv[d * 128 : (d + 1) * 128, :])
            xq0 = dma_xq_strip(0)
            xkv = []
            for mh in range(2):
                xkv_sb = xkvp.tile([128, 8, 1024], F16, tag="xkv", name=f"xkv{mh}")
                for d in range(8):
                    nc.sync.dma_start(
                        out=xkv_sb[:, d, :],
                        in_=xkvT[d * 128 : (d + 1) * 128, mh * 1024 : (mh + 1) * 1024],
                    )
                xkv.append(xkv_sb)
            mk_strip = dma_mask_strip(0)

            nc.vector.memset(V[:, :, :, DH : DH + 1], 1.0)
            project_q(xq0, 0)
            project_k_tile(wk_sb, xkv[0], 0, 0, 0)
            project_v_pair(wv_sb, xkv[0], 0, 0)
            project_v_pair(wv_sb, xkv[0], 0, 2)

            # PREPEND: carried tail thunks from the previous rep must not
            # push these past the m-iterations whose scores depend on them
            # (PE executes in queue order -> that would deadlock).
            def _kt(xh, g, mh, ms, act=False, wk_sb=wk_sb, xkv=xkv):
                return lambda: project_k_tile(wk_sb, xkv[xh], g, mh, ms, on_act=act)

            def _vp(xh, mh, mt0, wv_sb=wv_sb, xkv=xkv):
                return lambda: project_v_pair(wv_sb, xkv[xh], mh, mt0)

            pending[:0] = [
                _kt(0, 0, 0, 1),
                _vp(0, 0, 4),
                _vp(0, 0, 6),
                _kt(0, 1, 0, 0),
                _kt(0, 1, 0, 1),
                _kt(1, 0, 1, 0),
                _vp(1, 1, 0),
                _vp(1, 1, 2),
                _kt(1, 0, 1, 1),
                _vp(1, 1, 4),
                _vp(1, 1, 6),
                _kt(1, 1, 1, 0),
                _kt(1, 1, 1, 1),
            ]

            # ---------------- attention over n-strips --------------------
            for s in range(NS):
                mkd = mk_strip
                if s + 1 < NS:
                    mk_strip = dma_mask_strip(s + 1)

                ots = otsp.tile([DH + 1, HG, NSTRIP], F16, tag="ots", name=f"ots{s}")
                for g in range(2):
                    otg = otp.tile([DH + 1, 2, NSTRIP], F32, tag="ot", name=f"ot{s}{g}")
                    pe_mask = s % 2 == 1 and g == 0  # PE-load-balanced subset
                    for m in range(MT):
                        s2 = stp.tile([128, 2, NSTRIP], F32, tag="s2", name="s2")
                        for i in range(2):
                            nc.tensor.matmul(
                                s2[:, i, :],
                                KT[g][i * 64 : (i + 1) * 64, m * 128 : (m + 1) * 128],
                                QT[g][
                                    i * 64 : (i + 1) * 64,
                                    s * NSTRIP : (s + 1) * NSTRIP,
                                ],
                                start=True,
                                stop=not pe_mask,
                            )
                        if pe_mask:
                            for i in range(2):
                                nc.tensor.matmul(
                                    s2[:, i, :],
                                    ident6k,
                                    mkd[:, m, i, :],
                                    start=False,
                                    stop=True,
                                )
                        eta = etap.tile([128, 2, NSTRIP], F16, tag="eta", name="eta")
                        if "noexp" in _ABLATE:
                            nc.vector.tensor_copy(eta, s2)
                        else:
                            if pe_mask:
                                nc.scalar.activation(
                                    eta, s2, Exp, scale=SCALE, bias=neg750
                                )
                            else:
                                nc.scalar.activation(eta, s2, Exp, scale=SCALE)
                        if "nomask" not in _ABLATE and not pe_mask:
                            nc.vector.tensor_mul(eta, eta, mkd[:, m, :, :])
                        for i in range(2):
                            nc.tensor.matmul(
                                otg[:, i, :],
                                V[:, m, 2 * g + i, :],
                                eta[:, i, :],
                                start=(m == 0),
                                stop=(m == MT - 1),
                            )
                        if pending:
                            pending.pop(0)()
                    for i in range(2):
                        nc.vector.tensor_copy(
                            ots[:, 2 * g + i, :], otg[0 : DH + 1, i, :]
                        )
                    if g == 0 and s + 1 < NS:
                        xq_nxt = dma_xq_strip(s + 1)
                        pending.append(lambda xq=xq_nxt, ss=s + 1: project_q(xq, ss))

                emit_tail(s, ots)

        while pending:
            pending.pop(0)()

    _split_waits(nc)
    return nc


_SEL4 = np.zeros((4, 4, DH), dtype=np.float16)
for _h in range(4):
    _SEL4[_h, _h, :] = 1.0
_SEL4 = np.ascontiguousarray(_SEL4.reshape(4, 4 * DH))

_NC_CACHE = {}
_TRACE = False
_LAST_EXEC_NS = None


def _get_nc():
    if "nc" not in _NC_CACHE:
        _NC_CACHE["nc"] = build_nc()
    return _NC_CACHE["nc"]


def make_in_maps(inputs_q, inputs_kv, attention_mask, Wq, bq, Wk, bk, Wv, bv, Wp):
    """Per-core input dicts (shared by kernel() and test.py's bench)."""
    in_maps = []
    for c in range(NCORES):
        bidx, g = divmod(c, HG)
        cs = slice(g * CSL, (g + 1) * CSL)
        maskT = attention_mask[bidx, 0].T.astype(np.float16)  # [M, N]
        maskd = np.ascontiguousarray(
            np.repeat(maskT[:, None, :], 2, axis=1)
        )  # [M, 2, N]
        in_maps.append(
            {
                "xqT": np.ascontiguousarray(inputs_q[bidx].T.astype(np.float16)),
                "xkvT": np.ascontiguousarray(inputs_kv[bidx].T.astype(np.float16)),
                "maskdT": maskd,
                "wq": np.ascontiguousarray(Wq[:, cs].astype(np.float16)),
                "wk": np.ascontiguousarray(Wk[:, cs].astype(np.float16)),
                "wv": np.ascontiguousarray(Wv[:, cs].astype(np.float16)),
                "wp": np.ascontiguousarray(Wp[cs, :].astype(np.float16)),
                "bq2": np.ascontiguousarray(
                    bq[cs].astype(np.float32).reshape(2, 128).T
                ),
                "bk2": np.ascontiguousarray(
                    bk[cs].astype(np.float32).reshape(2, 128).T
                ),
                "bv1": np.ascontiguousarray(bv[cs].reshape(1, CSL).astype(np.float16)),
                "sel4in": _SEL4,
            }
        )
    return in_maps


def kernel(
    inputs_kv, inputs_q, attention_mask, Wq, bq, Wk, bk, Wv, bv, Wp, bp, **_unused
):
    inputs_kv = np.asarray(inputs_kv, dtype=np.float32)
    inputs_q = np.asarray(inputs_q, dtype=np.float32)
    attention_mask = np.asarray(attention_mask)
    bp = np.asarray(bp, dtype=np.float32)

    in_maps = make_in_maps(
        inputs_q, inputs_kv, attention_mask,
        np.asarray(Wq, dtype=np.float32), np.asarray(bq, dtype=np.float32),
        np.asarray(Wk, dtype=np.float32), np.asarray(bk, dtype=np.float32),
        np.asarray(Wv, dtype=np.float32), np.asarray(bv, dtype=np.float32),
        np.asarray(Wp, dtype=np.float32),
    )

    nc = _get_nc()
    res = run_bass_kernel_spmd(
        nc, in_maps, core_ids=list(range(NCORES)), trace=_TRACE
    )
    global _LAST_EXEC_NS
    _LAST_EXEC_NS = res.exec_time_ns

    out = np.zeros((B, N, D), dtype=np.float32)
    for c in range(NCORES):
        bidx = c // HG
        out[bidx] += res.results[c]["outp"].astype(np.float32)
    out += bp
    return out


# revision 16
# speedup vs baseline: 1.5186x; 1.5186x over previous
"""Trainium2 Bass kernel for nn_MultiHeadAttention (B=2, N=M=2048, D=1024, H=16).

Sharding: 8 cores = 2 batches x 4 head-groups (4 heads per core, tensor-parallel
over the head dim of Wq/Wk/Wv/Wp).  Each core computes a partial output
projection [N, D]; the host sums the 4 partials per batch and adds bp.

v3 dataflow (ACT-saturating pipeline; ACT's exp of the 4 x 2048 x 2048 score
matrix at 1 elem/lane/cycle is the hard floor, ~128us/core):
  - attention runs over 4 n-strips of 512.  Per (strip, g, m-chunk): two
    64-contraction score matmuls (heads 2g / 2g+1, PE row-tiles at partition
    bases 0/64 run them concurrently) into a double-buffered [128, 2, 512]
    PSUM pair; ONE ACT exp op over the full [128, 1024] with the 1/sqrt(dh)
    scale folded in; ONE DVE f16 mask multiply against a host-duplicated
    [m, 2, n] mask strip; two AV matmuls accumulating [65, 512] per head
    (ones-column = softmax denominator).
  - everything else rides in the PE/DVE slack under ACT via a thunk queue
    drained one item per m-iteration: the K/V/Q projections (prelude work
    interleaves into strip 0 so the first exp fires ~10us in; an up-front
    dummy exp preloads the ACT table during the DMA ramp), the Q^T
    projection for strip s+1, and strip s's tail (denominator gather,
    PE-transpose -> f16 reciprocal -> broadcast matmul -> in-place
    normalize, then the output projection as 4 x K=64 matmuls per [128,512]
    tile).  The queue carries across reps, so rep r's last tail overlaps
    rep r+1's prelude.
"""

import os
import numpy as np
from contextlib import ExitStack

import concourse.bass as bass
import concourse.tile as tile
from concourse import mybir
from concourse.bass_utils import run_bass_kernel_spmd
from concourse.vector_clock import ScopedClock
from concourse.masks import make_identity

B, N, M, D = 2, 2048, 2048, 1024
H = 16
DH = D // H  # 64
SCALE = DH ** -0.5
NCORES = 8
HG = 4            # heads per core
CSL = HG * DH     # 256 columns of Wq/Wk/Wv per core
F32 = mybir.dt.float32
F16 = mybir.dt.float16

NSTRIP = 512          # attention n-strip width
NS = N // NSTRIP      # 4 strips
MT = M // 128         # 16 m-chunks

_ABLATE = os.environ.get("ABLATE", "")  # comma-list: nomask, noexp

# ---------------------------------------------------------------------------
# walrus in this container rejects >1 sem wait per instruction; spread the
# extras across preceding same-engine NOPs (queues execute in order, so this
# is semantically identical).
_MAX_WAITS = 1


def _patched_drain_and_barrier(self, tick_clock, wait_clock):
    drain_inst = self.nc.sync.drain()
    wait_clock.add_sem_waits(
        drain_inst.ins, ScopedClock({None: tick_clock.global_clock})
    )
    si = drain_inst.ins.sync_info
    waits = list(si.on_wait or []) if si else []
    if len(waits) > _MAX_WAITS:
        si.on_wait = waits[:_MAX_WAITS]
        for i in range(_MAX_WAITS, len(waits), _MAX_WAITS):
            extra = self.nc.sync.drain()
            extra.ins.sync_info = mybir.SyncInfo(
                on_wait=waits[i : i + _MAX_WAITS], on_update=[]
            )
    self.nc.all_engine_barrier()
    assert self.sems is not None
    popped = self.nc._tile_sem_poison_stack.pop()
    assert popped is self._sem_poison
    self.nc.clear_and_free_semaphores(list(self.sems.allocated().values()))
    self.nc.all_engine_barrier()


tile.TileContext._drain_and_barrier = _patched_drain_and_barrier
# ---------------------------------------------------------------------------

Exp = mybir.ActivationFunctionType.Exp
Identity = mybir.ActivationFunctionType.Identity


def _split_waits(nc):
    n_split = 0
    for bb in nc.main_func.blocks:
        new_list = []
        for ins in bb.instructions:
            si = ins.sync_info
            if si is not None and si.on_wait and len(si.on_wait) > 1:
                waits = list(si.on_wait)
                for j, w in enumerate(waits[:-1]):
                    nop = mybir.InstNoOp(
                        name=f"{ins.name}-sw{j}",
                        engine=ins.engine,
                        sync_info=mybir.SyncInfo(on_wait=[w], on_update=[]),
                    )
                    new_list.append(nop)
                    n_split += 1
                si.on_wait = [waits[-1]]
            new_list.append(ins)
        bb.instructions = new_list
    return n_split


def build_nc(reps: int = 1) -> bass.Bass:
    nc = bass.Bass()

    xqT = nc.dram_tensor("xqT", [D, N], F16, kind="ExternalInput")
    xkvT = nc.dram_tensor("xkvT", [D, M], F16, kind="ExternalInput")
    maskdT = nc.dram_tensor("maskdT", [M, 2, N], F16, kind="ExternalInput")
    wq = nc.dram_tensor("wq", [D, CSL], F16, kind="ExternalInput")
    wk = nc.dram_tensor("wk", [D, CSL], F16, kind="ExternalInput")
    wv = nc.dram_tensor("wv", [D, CSL], F16, kind="ExternalInput")
    wp = nc.dram_tensor("wp", [CSL, D], F16, kind="ExternalInput")
    bq2 = nc.dram_tensor("bq2", [128, 2], F32, kind="ExternalInput")
    bk2 = nc.dram_tensor("bk2", [128, 2], F32, kind="ExternalInput")
    bv1 = nc.dram_tensor("bv1", [1, CSL], F16, kind="ExternalInput")
    sel4in = nc.dram_tensor("sel4in", [4, HG * DH], F16, kind="ExternalInput")
    outp = nc.dram_tensor("outp", [N, D], F16, kind="ExternalOutput")

    with ExitStack() as ctx:
        tc = ctx.enter_context(tile.TileContext(nc))

        consts = ctx.enter_context(tc.tile_pool(name="consts", bufs=1))
        ident = consts.tile([128, 128], F16)
        make_identity(nc, ident)
        # 6000*I: folds the mask into score PSUM (out += 6000*mask), paired
        # with bias=-750 inside the exp so masked entries underflow to 0.
        ident6k = consts.tile([128, 128], F16)
        make_identity(nc, ident6k)
        nc.vector.tensor_scalar_mul(ident6k, ident6k, 6000.0)
        ones_row = consts.tile([1, 128], F16)
        nc.vector.memset(ones_row, 1.0)
        sel4 = consts.tile([4, 4, DH], F16)
        nc.sync.dma_start(out=sel4, in_=sel4in[:, :])
        bq_sb = consts.tile([128, 2], F32)
        nc.sync.dma_start(out=bq_sb, in_=bq2[:, :])
        bk_sb = consts.tile([128, 2], F32)
        nc.sync.dma_start(out=bk_sb, in_=bk2[:, :])
        bv_sb = consts.tile([1, CSL], F16)
        nc.sync.dma_start(out=bv_sb, in_=bv1[:, :])
        # wp per head at partitions 0-63: [64, h, slice, 512]
        wp_sb = consts.tile([DH, HG, 2, 512], F16)
        for h in range(HG):
            nc.sync.dma_start(
                out=wp_sb[:, h, :, :], in_=wp[h * DH : (h + 1) * DH, :]
            )
        neg750 = consts.tile([128, 1], F32)
        nc.vector.memset(neg750, -750.0)
        # preload the exp table set while the first DMAs run
        warm = consts.tile([1, 2], F32)
        nc.vector.memset(warm, 0.0)
        nc.scalar.activation(warm, warm, Exp, scale=1.0)

        persist = ctx.enter_context(tc.tile_pool(name="persist", bufs=1))
        KT = [persist.tile([128, M], F16, tag=f"kt{g}", name=f"kt{g}") for g in range(2)]
        QT = [persist.tile([128, N], F16, tag=f"qt{g}", name=f"qt{g}") for g in range(2)]
        V = persist.tile([128, MT, HG, DH + 1], F16, tag="v")
        wq_sb = persist.tile([128, 8, CSL], F16, tag="wq")
        for d in range(8):
            nc.sync.dma_start(out=wq_sb[:, d, :], in_=wq[d * 128 : (d + 1) * 128, :])

        maskp = ctx.enter_context(tc.tile_pool(name="maskp", bufs=2))
        xqp = ctx.enter_context(tc.tile_pool(name="xqp", bufs=2))
        xkvp = ctx.enter_context(tc.tile_pool(name="xkvp", bufs=2))
        wkvp = ctx.enter_context(tc.tile_pool(name="wkvp", bufs=1))
        etap = ctx.enter_context(tc.tile_pool(name="etap", bufs=4))
        otsp = ctx.enter_context(tc.tile_pool(name="otsp", bufs=2))
        obp = ctx.enter_context(tc.tile_pool(name="obp", bufs=2))
        nrmp = ctx.enter_context(tc.tile_pool(name="nrmp", bufs=2))

        stp = ctx.enter_context(tc.tile_pool(name="stp", bufs=2, space="PSUM"))
        otp = ctx.enter_context(tc.tile_pool(name="otp", bufs=1, space="PSUM"))
        tailp = ctx.enter_context(tc.tile_pool(name="tailp", bufs=2, space="PSUM"))

        pending = []  # emission thunks drained one per m-iteration

        def dma_xq_strip(s):
            xq_sb = xqp.tile([128, 8, NSTRIP], F16, tag="xq", name=f"xq{s}")
            for d in range(8):
                nc.sync.dma_start(
                    out=xq_sb[:, d, :],
                    in_=xqT[d * 128 : (d + 1) * 128, s * NSTRIP : (s + 1) * NSTRIP],
                )
            return xq_sb

        def dma_mask_strip(s):
            mkd = maskp.tile([128, MT, 2, 512], F16, tag="mk", name=f"mk{s}")
            for m in range(MT):
                nc.sync.dma_start(
                    out=mkd[:, m, :, :],
                    in_=maskdT[
                        m * 128 : (m + 1) * 128, :, s * NSTRIP : (s + 1) * NSTRIP
                    ],
                )
            return mkd

        def project_q(xq_sb, s, on_act=False):
            for g in range(2):
                ps = tailp.tile([128, NSTRIP], F32, tag="t", name=f"qps{s}{g}")
                for d in range(8):
                    nc.tensor.matmul(
                        ps,
                        wq_sb[:, d, g * 128 : (g + 1) * 128],
                        xq_sb[:, d, :],
                        start=(d == 0),
                        stop=(d == 7),
                    )
                dst = QT[g][:, s * NSTRIP : (s + 1) * NSTRIP]
                if on_act:
                    nc.scalar.activation(dst, ps, Identity, bias=bq_sb[:, g : g + 1])
                else:
                    nc.vector.tensor_scalar_add(dst, ps, bq_sb[:, g : g + 1])

        def project_k_tile(wk_sb, xkv_sb, g, mh, ms, on_act=False):
            ps = tailp.tile([128, 512], F32, tag="t", name="kps")
            for d in range(8):
                nc.tensor.matmul(
                    ps,
                    wk_sb[:, d, g * 128 : (g + 1) * 128],
                    xkv_sb[:, d, ms * 512 : (ms + 1) * 512],
                    start=(d == 0),
                    stop=(d == 7),
                )
            dst = KT[g][:, mh * 1024 + ms * 512 : mh * 1024 + (ms + 1) * 512]
            if on_act:
                nc.scalar.activation(dst, ps, Identity, bias=bk_sb[:, g : g + 1])
            else:
                nc.vector.tensor_scalar_add(dst, ps, bk_sb[:, g : g + 1])

        def project_v_pair(wv_sb, xkv_sb, mh, mt0):
            for mt in (mt0, mt0 + 1):
                vp = tailp.tile([128, HG, DH], F32, tag="t", name="vp")
                for d in range(8):
                    nc.tensor.matmul(
                        vp,
                        xkv_sb[:, d, mt * 128 : (mt + 1) * 128],
                        wv_sb[:, d, :],
                        start=(d == 0),
                        stop=False,
                    )
                nc.tensor.matmul(vp, ones_row, bv_sb, start=False, stop=True)
                nc.vector.tensor_copy(V[:, mh * 8 + mt, :, 0:DH], vp)

        def emit_tail(s, ots):
            """Normalize + output projection for strip s, as thunks."""

            def norm_chain(s=s, ots=ots):
                rowsums = nrmp.tile([4, NSTRIP], F16, tag="rs", name="rs")
                nc.sync.dma_start(out=rowsums, in_=ots[DH : DH + 1, :, :])
                rsT = tailp.tile([128, 16], F16, tag="t", name="rsT")
                for b in range(4):
                    nc.tensor.transpose(
                        rsT[:, 4 * b : 4 * b + 4],
                        rowsums[:, b * 128 : (b + 1) * 128],
                        ident[0:4, 0:4],
                    )
                recipT = nrmp.tile([128, 16], F16, tag="rcp", name="recipT")
                with nc.allow_low_precision(
                    reason="softmax denominators are O(1e3); f16 recip gives "
                    "~5e-4 rel err, well inside the output tolerance"
                ):
                    nc.vector.reciprocal(recipT, rsT)
                rrps = tailp.tile([4, NSTRIP], F16, tag="t", name="rrps")
                for b in range(4):
                    nc.tensor.transpose(
                        rrps[:, b * 128 : (b + 1) * 128],
                        recipT[:, 4 * b : 4 * b + 4],
                        ident,
                    )
                recrows = nrmp.tile([4, NSTRIP], F16, tag="rr", name="recrows")
                nc.vector.tensor_copy(recrows, rrps)
                emit_tail.recrows = recrows

            def norm_heads(h0, s=s, ots=ots):
                recrows = emit_tail.recrows
                for h in (h0, h0 + 1):
                    rps = tailp.tile([DH, NSTRIP], F32, tag="t", name="rps")
                    nc.tensor.matmul(rps, sel4[:, h, :], recrows)
                    nc.vector.tensor_mul(ots[0:DH, h, :], ots[0:DH, h, :], rps)

            def out_tile(t, s=s, ots=ots):
                ob = obp.tile([128, 2, 512], F16, tag="ob", name="ob")
                for sl in range(2):
                    po = tailp.tile([128, 512], F32, tag="t", name="po")
                    for h in range(HG):
                        nc.tensor.matmul(
                            po,
                            ots[0:DH, h, t * 128 : (t + 1) * 128],
                            wp_sb[:, h, sl, :],
                            start=(h == 0),
                            stop=(h == HG - 1),
                        )
                    nc.vector.tensor_copy(ob[:, sl, :], po)
                row = s * NSTRIP + t * 128
                nc.sync.dma_start(out=outp[row : row + 128, :], in_=ob)

            pending.append(norm_chain)
            pending.append(lambda: norm_heads(0))
            pending.append(lambda: norm_heads(2))
            for t in range(NSTRIP // 128):
                pending.append(lambda t=t: out_tile(t))

        for _rep in range(reps):
            # ---- prelude: DMAs + enough projections to start strip 0 ----
            wk_sb = wkvp.tile([128, 8, CSL], F16, tag="wk", name="wk_sb")
            wv_sb = wkvp.tile([128, 8, CSL], F16, tag="wv", name="wv_sb")
            for d in range(8):
                nc.sync.dma_start(out=wk_sb[:, d, :], in_=wk[d * 128 : (d + 1) * 128, :])
                nc.sync.dma_start(out=wv_sb[:, d, :], in_=wv[d * 128 : (d + 1) * 128, :])
            xq0 = dma_xq_strip(0)
            xkv = []
            for mh in range(2):
                xkv_sb = xkvp.tile([128, 8, 1024], F16, tag="xkv", name=f"xkv{mh}")
                for d in range(8):
                    nc.sync.dma_start(
                        out=xkv_sb[:, d, :],
                        in_=xkvT[d * 128 : (d + 1) * 128, mh * 1024 : (mh + 1) * 1024],
                    )
                xkv.append(xkv_sb)
            mk_strip = dma_mask_strip(0)

            nc.vector.memset(V[:, :, :, DH : DH + 1], 1.0)
            project_q(xq0, 0)
            project_k_tile(wk_sb, xkv[0], 0, 0, 0)
            project_v_pair(wv_sb, xkv[0], 0, 0)
            project_v_pair(wv_sb, xkv[0], 0, 2)

            # PREPEND: carried tail thunks from the previous rep must not
            # push these past the m-iterations whose scores depend on them
            # (PE executes in queue order -> that would deadlock).
            def _kt(xh, g, mh, ms, act=False, wk_sb=wk_sb, xkv=xkv):
                return lambda: project_k_tile(wk_sb, xkv[xh], g, mh, ms, on_act=act)

            def _vp(xh, mh, mt0, wv_sb=wv_sb, xkv=xkv):
                return lambda: project_v_pair(wv_sb, xkv[xh], mh, mt0)

            pending[:0] = [
                _kt(0, 0, 0, 1),
                _vp(0, 0, 4),
                _vp(0, 0, 6),
                _kt(0, 1, 0, 0),
                _kt(0, 1, 0, 1),
                _kt(1, 0, 1, 0),
                _vp(1, 1, 0),
                _vp(1, 1, 2),
                _kt(1, 0, 1, 1),
                _vp(1, 1, 4),
                _vp(1, 1, 6),
                _kt(1, 1, 1, 0),
                _kt(1, 1, 1, 1),
            ]

            # ---------------- attention over n-strips --------------------
            for s in range(NS):
                mkd = mk_strip
                if s + 1 < NS:
                    mk_strip = dma_mask_strip(s + 1)

                ots = otsp.tile([DH + 1, HG, NSTRIP], F16, tag="ots", name=f"ots{s}")
                for g in range(2):
                    otg = otp.tile([DH + 1, 2, NSTRIP], F32, tag="ot", name=f"ot{s}{g}")
                    pe_mask = False  # PE-bias mask path regressed on HW; keep DVE mask
                    for m in range(MT):
                        s2 = stp.tile([128, 2, NSTRIP], F32, tag="s2", name="s2")
                        for i in range(2):
                            nc.tensor.matmul(
                                s2[:, i, :],
                                KT[g][i * 64 : (i + 1) * 64, m * 128 : (m + 1) * 128],
                                QT[g][
                                    i * 64 : (i + 1) * 64,
                                    s * NSTRIP : (s + 1) * NSTRIP,
                                ],
                                start=True,
                                stop=not pe_mask,
                            )
                        if pe_mask:
                            for i in range(2):
                                nc.tensor.matmul(
                                    s2[:, i, :],
                                    ident6k,
                                    mkd[:, m, i, :],
                                    start=False,
                                    stop=True,
                                )
                        eta = etap.tile([128, 2, NSTRIP], F16, tag="eta", name="eta")
                        if "noexp" in _ABLATE:
                            nc.vector.tensor_copy(eta, s2)
                        else:
                            if pe_mask:
                                nc.scalar.activation(
                                    eta, s2, Exp, scale=SCALE, bias=neg750
                                )
                            else:
                                nc.scalar.activation(eta, s2, Exp, scale=SCALE)
                        if "nomask" not in _ABLATE and not pe_mask:
                            nc.vector.tensor_mul(eta, eta, mkd[:, m, :, :])
                        for i in range(2):
                            nc.tensor.matmul(
                                otg[:, i, :],
                                V[:, m, 2 * g + i, :],
                                eta[:, i, :],
                                start=(m == 0),
                                stop=(m == MT - 1),
                            )
                        if pending:
                            pending.pop(0)()
                    for i in range(2):
                        nc.vector.tensor_copy(
                            ots[:, 2 * g + i, :], otg[0 : DH + 1, i, :]
                        )
                    if g == 0 and s + 1 < NS:
                        xq_nxt = dma_xq_strip(s + 1)
                        pending.append(lambda xq=xq_nxt, ss=s + 1: project_q(xq, ss))

                emit_tail(s, ots)

        while pending:
            pending.pop(0)()

    _split_waits(nc)
    return nc


_SEL4 = np.zeros((4, 4, DH), dtype=np.float16)
for _h in range(4):
    _SEL4[_h, _h, :] = 1.0
_SEL4 = np.ascontiguousarray(_SEL4.reshape(4, 4 * DH))

_NC_CACHE = {}
_TRACE = False
_LAST_EXEC_NS = None


def _get_nc():
    if "nc" not in _NC_CACHE:
        _NC_CACHE["nc"] = build_nc()
    return _NC_CACHE["nc"]


def make_in_maps(inputs_q, inputs_kv, attention_mask, Wq, bq, Wk, bk, Wv, bv, Wp):
    """Per-core input dicts (shared by kernel() and test.py's bench)."""
    in_maps = []
    for c in range(NCORES):
        bidx, g = divmod(c, HG)
        cs = slice(g * CSL, (g + 1) * CSL)
        maskT = attention_mask[bidx, 0].T.astype(np.float16)  # [M, N]
        maskd = np.ascontiguousarray(
            np.repeat(maskT[:, None, :], 2, axis=1)
        )  # [M, 2, N]
        in_maps.append(
            {
                "xqT": np.ascontiguousarray(inputs_q[bidx].T.astype(np.float16)),
                "xkvT": np.ascontiguousarray(inputs_kv[bidx].T.astype(np.float16)),
                "maskdT": maskd,
                "wq": np.ascontiguousarray(Wq[:, cs].astype(np.float16)),
                "wk": np.ascontiguousarray(Wk[:, cs].astype(np.float16)),
                "wv": np.ascontiguousarray(Wv[:, cs].astype(np.float16)),
                "wp": np.ascontiguousarray(Wp[cs, :].astype(np.float16)),
                "bq2": np.ascontiguousarray(
                    bq[cs].astype(np.float32).reshape(2, 128).T
                ),
                "bk2": np.ascontiguousarray(
                    bk[cs].astype(np.float32).reshape(2, 128).T
                ),
                "bv1": np.ascontiguousarray(bv[cs].reshape(1, CSL).astype(np.float16)),
                "sel4in": _SEL4,
            }
        )
    return in_maps


def kernel(
    inputs_kv, inputs_q, attention_mask, Wq, bq, Wk, bk, Wv, bv, Wp, bp, **_unused
):
    inputs_kv = np.asarray(inputs_kv, dtype=np.float32)
    inputs_q = np.asarray(inputs_q, dtype=np.float32)
    attention_mask = np.asarray(attention_mask)
    bp = np.asarray(bp, dtype=np.float32)

    in_maps = make_in_maps(
        inputs_q, inputs_kv, attention_mask,
        np.asarray(Wq, dtype=np.float32), np.asarray(bq, dtype=np.float32),
        np.asarray(Wk, dtype=np.float32), np.asarray(bk, dtype=np.float32),
        np.asarray(Wv, dtype=np.float32), np.asarray(bv, dtype=np.float32),
        np.asarray(Wp, dtype=np.float32),
    )

    nc = _get_nc()
    res = run_bass_kernel_spmd(
        nc, in_maps, core_ids=list(range(NCORES)), trace=_TRACE
    )
    global _LAST_EXEC_NS
    _LAST_EXEC_NS = res.exec_time_ns

    out = np.zeros((B, N, D), dtype=np.float32)
    for c in range(NCORES):
        bidx = c // HG
        out[bidx] += res.results[c]["outp"].astype(np.float32)
    out += bp
    return out


# revision 17
# speedup vs baseline: 1.5394x; 1.0137x over previous
"""Trainium2 Bass kernel for nn_MultiHeadAttention (B=2, N=M=2048, D=1024, H=16).

Sharding: 8 cores = 2 batches x 4 head-groups (4 heads per core, tensor-parallel
over the head dim of Wq/Wk/Wv/Wp).  Each core computes a partial output
projection [N, D]; the host sums the 4 partials per batch and adds bp.

Dataflow (ACT-saturating pipeline; ACT's exp of the 4 x 2048 x 2048 score
matrix at 1 elem/lane/cycle @1.2GHz is the hard floor, ~128us/core, and the
scalar engine does NOTHING else in steady state):
  - attention runs over 4 n-strips of 512.  Per (strip, g, m-chunk): two
    64-contraction score matmuls (heads 2g / 2g+1, PE row-tiles at partition
    bases 0/64 run them concurrently) into a double-buffered [128, 2, 512]
    PSUM pair; ONE ACT exp op over the full [128, 1024] with the 1/sqrt(dh)
    scale folded in; ONE DVE f16 mask multiply against a host-duplicated
    [m, 2, n] mask strip; two AV matmuls accumulating [65, 512] per head
    (ones-column = softmax denominator).
  - everything else rides in the PE/DVE slack under ACT via a thunk queue
    drained one item per m-iteration: the K/V/Q projections (prelude work
    interleaves into strip 0 so the first exp fires ~10us in; an up-front
    dummy exp preloads the ACT table during the DMA ramp), the Q^T
    projection for strip s+1, and strip s's tail (denominator gather,
    PE-transpose -> f16 reciprocal -> broadcast matmul -> in-place
    normalize, then the output projection as 4 x K=64 matmuls per [128,512]
    tile).  The queue carries across reps, so rep r's last tail overlaps
    rep r+1's prelude.
"""

import os
import numpy as np
from contextlib import ExitStack

import concourse.bass as bass
import concourse.tile as tile
from concourse import mybir
from concourse.bass_utils import run_bass_kernel_spmd
from concourse.vector_clock import ScopedClock
from concourse.masks import make_identity

B, N, M, D = 2, 2048, 2048, 1024
H = 16
DH = D // H  # 64
SCALE = DH ** -0.5
NCORES = 8
HG = 4            # heads per core
CSL = HG * DH     # 256 columns of Wq/Wk/Wv per core
F32 = mybir.dt.float32
F16 = mybir.dt.float16

NSTRIP = 512          # attention n-strip width
NS = N // NSTRIP      # 4 strips
MT = M // 128         # 16 m-chunks

_ABLATE = os.environ.get("ABLATE", "")  # comma-list: nomask, noexp

# ---------------------------------------------------------------------------
# walrus in this container rejects >1 sem wait per instruction; spread the
# extras across preceding same-engine NOPs (queues execute in order, so this
# is semantically identical).
_MAX_WAITS = 1


def _patched_drain_and_barrier(self, tick_clock, wait_clock):
    drain_inst = self.nc.sync.drain()
    wait_clock.add_sem_waits(
        drain_inst.ins, ScopedClock({None: tick_clock.global_clock})
    )
    si = drain_inst.ins.sync_info
    waits = list(si.on_wait or []) if si else []
    if len(waits) > _MAX_WAITS:
        si.on_wait = waits[:_MAX_WAITS]
        for i in range(_MAX_WAITS, len(waits), _MAX_WAITS):
            extra = self.nc.sync.drain()
            extra.ins.sync_info = mybir.SyncInfo(
                on_wait=waits[i : i + _MAX_WAITS], on_update=[]
            )
    self.nc.all_engine_barrier()
    assert self.sems is not None
    popped = self.nc._tile_sem_poison_stack.pop()
    assert popped is self._sem_poison
    self.nc.clear_and_free_semaphores(list(self.sems.allocated().values()))
    self.nc.all_engine_barrier()


tile.TileContext._drain_and_barrier = _patched_drain_and_barrier
# ---------------------------------------------------------------------------

Exp = mybir.ActivationFunctionType.Exp
Identity = mybir.ActivationFunctionType.Identity


def _split_waits(nc):
    n_split = 0
    for bb in nc.main_func.blocks:
        new_list = []
        for ins in bb.instructions:
            si = ins.sync_info
            if si is not None and si.on_wait and len(si.on_wait) > 1:
                waits = list(si.on_wait)
                for j, w in enumerate(waits[:-1]):
                    nop = mybir.InstNoOp(
                        name=f"{ins.name}-sw{j}",
                        engine=ins.engine,
                        sync_info=mybir.SyncInfo(on_wait=[w], on_update=[]),
                    )
                    new_list.append(nop)
                    n_split += 1
                si.on_wait = [waits[-1]]
            new_list.append(ins)
        bb.instructions = new_list
    return n_split


def build_nc(reps: int = 1) -> bass.Bass:
    nc = bass.Bass()

    xqT = nc.dram_tensor("xqT", [D, N], F16, kind="ExternalInput")
    xkvT = nc.dram_tensor("xkvT", [D, M], F16, kind="ExternalInput")
    maskdT = nc.dram_tensor("maskdT", [M, 2, N], F16, kind="ExternalInput")
    wq = nc.dram_tensor("wq", [D, CSL], F16, kind="ExternalInput")
    wk = nc.dram_tensor("wk", [D, CSL], F16, kind="ExternalInput")
    wv = nc.dram_tensor("wv", [D, CSL], F16, kind="ExternalInput")
    wp = nc.dram_tensor("wp", [CSL, D], F16, kind="ExternalInput")
    bq2 = nc.dram_tensor("bq2", [128, 2], F32, kind="ExternalInput")
    bk2 = nc.dram_tensor("bk2", [128, 2], F32, kind="ExternalInput")
    bv1 = nc.dram_tensor("bv1", [1, CSL], F16, kind="ExternalInput")
    sel4in = nc.dram_tensor("sel4in", [4, HG * DH], F16, kind="ExternalInput")
    outp = nc.dram_tensor("outp", [N, D], F16, kind="ExternalOutput")

    with ExitStack() as ctx:
        tc = ctx.enter_context(tile.TileContext(nc))

        consts = ctx.enter_context(tc.tile_pool(name="consts", bufs=1))
        ident = consts.tile([128, 128], F16)
        make_identity(nc, ident)
        # 6000*I: folds the mask into score PSUM (out += 6000*mask), paired
        # with bias=-750 inside the exp so masked entries underflow to 0.
        ident6k = consts.tile([128, 128], F16)
        make_identity(nc, ident6k)
        nc.vector.tensor_scalar_mul(ident6k, ident6k, 6000.0)
        ones_row = consts.tile([1, 128], F16)
        nc.vector.memset(ones_row, 1.0)
        sel4 = consts.tile([4, 4, DH], F16)
        nc.sync.dma_start(out=sel4, in_=sel4in[:, :])
        bq_sb = consts.tile([128, 2], F32)
        nc.sync.dma_start(out=bq_sb, in_=bq2[:, :])
        bk_sb = consts.tile([128, 2], F32)
        nc.sync.dma_start(out=bk_sb, in_=bk2[:, :])
        bv_sb = consts.tile([1, CSL], F16)
        nc.sync.dma_start(out=bv_sb, in_=bv1[:, :])
        # wp per head at partitions 0-63: [64, h, slice, 512]
        wp_sb = consts.tile([DH, HG, 2, 512], F16)
        for h in range(HG):
            nc.sync.dma_start(
                out=wp_sb[:, h, :, :], in_=wp[h * DH : (h + 1) * DH, :]
            )
        neg750 = consts.tile([128, 1], F32)
        nc.vector.memset(neg750, -750.0)
        # preload the exp table set while the first DMAs run
        warm = consts.tile([1, 2], F32)
        nc.vector.memset(warm, 0.0)
        nc.scalar.activation(warm, warm, Exp, scale=1.0)

        persist = ctx.enter_context(tc.tile_pool(name="persist", bufs=1))
        KT = [persist.tile([128, M], F16, tag=f"kt{g}", name=f"kt{g}") for g in range(2)]
        QT = [persist.tile([128, N], F16, tag=f"qt{g}", name=f"qt{g}") for g in range(2)]
        V = persist.tile([128, MT, HG, DH + 1], F16, tag="v")
        wq_sb = persist.tile([128, 8, CSL], F16, tag="wq")
        for d in range(8):
            nc.sync.dma_start(out=wq_sb[:, d, :], in_=wq[d * 128 : (d + 1) * 128, :])

        maskp = ctx.enter_context(tc.tile_pool(name="maskp", bufs=2))
        xqp = ctx.enter_context(tc.tile_pool(name="xqp", bufs=2))
        xkvp = ctx.enter_context(tc.tile_pool(name="xkvp", bufs=2))
        wkvp = ctx.enter_context(tc.tile_pool(name="wkvp", bufs=1))
        etap = ctx.enter_context(tc.tile_pool(name="etap", bufs=4))
        otsp = ctx.enter_context(tc.tile_pool(name="otsp", bufs=2))
        obp = ctx.enter_context(tc.tile_pool(name="obp", bufs=2))
        nrmp = ctx.enter_context(tc.tile_pool(name="nrmp", bufs=2))

        stp = ctx.enter_context(tc.tile_pool(name="stp", bufs=2, space="PSUM"))
        otp = ctx.enter_context(tc.tile_pool(name="otp", bufs=1, space="PSUM"))
        tailp = ctx.enter_context(tc.tile_pool(name="tailp", bufs=2, space="PSUM"))

        pending = []  # emission thunks drained one per m-iteration

        def dma_xq_strip(s):
            xq_sb = xqp.tile([128, 8, NSTRIP], F16, tag="xq", name=f"xq{s}")
            for d in range(8):
                nc.sync.dma_start(
                    out=xq_sb[:, d, :],
                    in_=xqT[d * 128 : (d + 1) * 128, s * NSTRIP : (s + 1) * NSTRIP],
                )
            return xq_sb

        def dma_mask_strip(s):
            mkd = maskp.tile([128, MT, 2, 512], F16, tag="mk", name=f"mk{s}")
            for m in range(MT):
                nc.sync.dma_start(
                    out=mkd[:, m, :, :],
                    in_=maskdT[
                        m * 128 : (m + 1) * 128, :, s * NSTRIP : (s + 1) * NSTRIP
                    ],
                )
            return mkd

        def project_q(xq_sb, s, on_act=False):
            for g in range(2):
                ps = tailp.tile([128, NSTRIP], F32, tag="t", name=f"qps{s}{g}")
                for d in range(8):
                    nc.tensor.matmul(
                        ps,
                        wq_sb[:, d, g * 128 : (g + 1) * 128],
                        xq_sb[:, d, :],
                        start=(d == 0),
                        stop=(d == 7),
                    )
                dst = QT[g][:, s * NSTRIP : (s + 1) * NSTRIP]
                if on_act:
                    nc.scalar.activation(dst, ps, Identity, bias=bq_sb[:, g : g + 1])
                else:
                    nc.vector.tensor_scalar_add(dst, ps, bq_sb[:, g : g + 1])

        def project_k_tile(wk_sb, xkv_sb, g, mh, ms, on_act=False):
            ps = tailp.tile([128, 512], F32, tag="t", name="kps")
            for d in range(8):
                nc.tensor.matmul(
                    ps,
                    wk_sb[:, d, g * 128 : (g + 1) * 128],
                    xkv_sb[:, d, ms * 512 : (ms + 1) * 512],
                    start=(d == 0),
                    stop=(d == 7),
                )
            dst = KT[g][:, mh * 1024 + ms * 512 : mh * 1024 + (ms + 1) * 512]
            if on_act:
                nc.scalar.activation(dst, ps, Identity, bias=bk_sb[:, g : g + 1])
            else:
                nc.vector.tensor_scalar_add(dst, ps, bk_sb[:, g : g + 1])

        def project_v_pair(wv_sb, xkv_sb, mh, mt0):
            for mt in (mt0, mt0 + 1):
                vp = tailp.tile([128, HG, DH], F32, tag="t", name="vp")
                for d in range(8):
                    nc.tensor.matmul(
                        vp,
                        xkv_sb[:, d, mt * 128 : (mt + 1) * 128],
                        wv_sb[:, d, :],
                        start=(d == 0),
                        stop=False,
                    )
                nc.tensor.matmul(vp, ones_row, bv_sb, start=False, stop=True)
                nc.vector.tensor_copy(V[:, mh * 8 + mt, :, 0:DH], vp)

        def emit_tail(s, ots):
            """Normalize + output projection for strip s, as thunks."""

            def norm_chain(s=s, ots=ots):
                rowsums = nrmp.tile([4, NSTRIP], F16, tag="rs", name="rs")
                nc.sync.dma_start(out=rowsums, in_=ots[DH : DH + 1, :, :])
                rsT = tailp.tile([128, 16], F16, tag="t", name="rsT")
                for b in range(4):
                    nc.tensor.transpose(
                        rsT[:, 4 * b : 4 * b + 4],
                        rowsums[:, b * 128 : (b + 1) * 128],
                        ident[0:4, 0:4],
                    )
                recipT = nrmp.tile([128, 16], F16, tag="rcp", name="recipT")
                with nc.allow_low_precision(
                    reason="softmax denominators are O(1e3); f16 recip gives "
                    "~5e-4 rel err, well inside the output tolerance"
                ):
                    nc.vector.reciprocal(recipT, rsT)
                rrps = tailp.tile([4, NSTRIP], F16, tag="t", name="rrps")
                for b in range(4):
                    nc.tensor.transpose(
                        rrps[:, b * 128 : (b + 1) * 128],
                        recipT[:, 4 * b : 4 * b + 4],
                        ident,
                    )
                recrows = nrmp.tile([4, NSTRIP], F16, tag="rr", name="recrows")
                nc.vector.tensor_copy(recrows, rrps)
                emit_tail.recrows = recrows

            def norm_heads(h0, s=s, ots=ots):
                recrows = emit_tail.recrows
                for h in (h0, h0 + 1):
                    rps = tailp.tile([DH, NSTRIP], F32, tag="t", name="rps")
                    nc.tensor.matmul(rps, sel4[:, h, :], recrows)
                    nc.vector.tensor_mul(ots[0:DH, h, :], ots[0:DH, h, :], rps)

            def out_tile(t, s=s, ots=ots):
                ob = obp.tile([128, 2, 512], F16, tag="ob", name="ob")
                for sl in range(2):
                    po = tailp.tile([128, 512], F32, tag="t", name="po")
                    for h in range(HG):
                        nc.tensor.matmul(
                            po,
                            ots[0:DH, h, t * 128 : (t + 1) * 128],
                            wp_sb[:, h, sl, :],
                            start=(h == 0),
                            stop=(h == HG - 1),
                        )
                    nc.vector.tensor_copy(ob[:, sl, :], po)
                row = s * NSTRIP + t * 128
                nc.sync.dma_start(out=outp[row : row + 128, :], in_=ob)

            pending.append(norm_chain)
            pending.append(lambda: norm_heads(0))
            pending.append(lambda: norm_heads(2))
            for t in range(NSTRIP // 128):
                pending.append(lambda t=t: out_tile(t))

        for _rep in range(reps):
            # ---- prelude: DMAs + enough projections to start strip 0 ----
            wk_sb = wkvp.tile([128, 8, CSL], F16, tag="wk", name="wk_sb")
            wv_sb = wkvp.tile([128, 8, CSL], F16, tag="wv", name="wv_sb")
            for d in range(8):
                nc.sync.dma_start(out=wk_sb[:, d, :], in_=wk[d * 128 : (d + 1) * 128, :])
                nc.sync.dma_start(out=wv_sb[:, d, :], in_=wv[d * 128 : (d + 1) * 128, :])
            xq0 = dma_xq_strip(0)
            xkv = []
            for mh in range(2):
                xkv_sb = xkvp.tile([128, 8, 1024], F16, tag="xkv", name=f"xkv{mh}")
                for d in range(8):
                    nc.sync.dma_start(
                        out=xkv_sb[:, d, :],
                        in_=xkvT[d * 128 : (d + 1) * 128, mh * 1024 : (mh + 1) * 1024],
                    )
                xkv.append(xkv_sb)
            mk_strip = dma_mask_strip(0)

            nc.vector.memset(V[:, :, :, DH : DH + 1], 1.0)
            project_q(xq0, 0)
            project_k_tile(wk_sb, xkv[0], 0, 0, 0)
            project_v_pair(wv_sb, xkv[0], 0, 0)
            project_v_pair(wv_sb, xkv[0], 0, 2)

            # PREPEND: carried tail thunks from the previous rep must not
            # push these past the m-iterations whose scores depend on them
            # (PE executes in queue order -> that would deadlock).
            def _kt(xh, g, mh, ms, act=False, wk_sb=wk_sb, xkv=xkv):
                return lambda: project_k_tile(wk_sb, xkv[xh], g, mh, ms, on_act=act)

            def _vp(xh, mh, mt0, wv_sb=wv_sb, xkv=xkv):
                return lambda: project_v_pair(wv_sb, xkv[xh], mh, mt0)

            pending[:0] = [
                _kt(0, 0, 0, 1),
                _vp(0, 0, 4),
                _vp(0, 0, 6),
                _kt(0, 1, 0, 0),
                _kt(0, 1, 0, 1),
                _kt(1, 0, 1, 0),
                _vp(1, 1, 0),
                _vp(1, 1, 2),
                _kt(1, 0, 1, 1),
                _vp(1, 1, 4),
                _vp(1, 1, 6),
                _kt(1, 1, 1, 0),
                _kt(1, 1, 1, 1),
            ]

            # ---------------- attention over n-strips --------------------
            for s in range(NS):
                mkd = mk_strip
                if s + 1 < NS:
                    mk_strip = dma_mask_strip(s + 1)

                ots = otsp.tile([DH + 1, HG, NSTRIP], F16, tag="ots", name=f"ots{s}")
                for g in range(2):
                    otg = otp.tile([DH + 1, 2, NSTRIP], F32, tag="ot", name=f"ot{s}{g}")
                    pe_mask = False  # PE-bias mask path regressed on HW; keep DVE mask
                    for m in range(MT):
                        s2 = stp.tile([128, 2, NSTRIP], F32, tag="s2", name="s2")
                        for i in range(2):
                            nc.tensor.matmul(
                                s2[:, i, :],
                                KT[g][i * 64 : (i + 1) * 64, m * 128 : (m + 1) * 128],
                                QT[g][
                                    i * 64 : (i + 1) * 64,
                                    s * NSTRIP : (s + 1) * NSTRIP,
                                ],
                                start=True,
                                stop=not pe_mask,
                            )
                        if pe_mask:
                            for i in range(2):
                                nc.tensor.matmul(
                                    s2[:, i, :],
                                    ident6k,
                                    mkd[:, m, i, :],
                                    start=False,
                                    stop=True,
                                )
                        eta = etap.tile([128, 2, NSTRIP], F16, tag="eta", name="eta")
                        if "noexp" in _ABLATE:
                            nc.vector.tensor_copy(eta, s2)
                        else:
                            if pe_mask:
                                nc.scalar.activation(
                                    eta, s2, Exp, scale=SCALE, bias=neg750
                                )
                            else:
                                nc.scalar.activation(eta, s2, Exp, scale=SCALE)
                        if "nomask" not in _ABLATE and not pe_mask:
                            nc.vector.tensor_mul(eta, eta, mkd[:, m, :, :])
                        for i in range(2):
                            nc.tensor.matmul(
                                otg[:, i, :],
                                V[:, m, 2 * g + i, :],
                                eta[:, i, :],
                                start=(m == 0),
                                stop=(m == MT - 1),
                            )
                        if pending:
                            pending.pop(0)()
                    for i in range(2):
                        nc.vector.tensor_copy(
                            ots[:, 2 * g + i, :], otg[0 : DH + 1, i, :]
                        )
                    if g == 0 and s + 1 < NS:
                        xq_nxt = dma_xq_strip(s + 1)
                        pending.append(lambda xq=xq_nxt, ss=s + 1: project_q(xq, ss))

                emit_tail(s, ots)

        while pending:
            pending.pop(0)()

    _split_waits(nc)
    return nc


_SEL4 = np.zeros((4, 4, DH), dtype=np.float16)
for _h in range(4):
    _SEL4[_h, _h, :] = 1.0
_SEL4 = np.ascontiguousarray(_SEL4.reshape(4, 4 * DH))

_NC_CACHE = {}
_TRACE = False
_LAST_EXEC_NS = None


def _get_nc():
    if "nc" not in _NC_CACHE:
        _NC_CACHE["nc"] = build_nc()
    return _NC_CACHE["nc"]


def make_in_maps(inputs_q, inputs_kv, attention_mask, Wq, bq, Wk, bk, Wv, bv, Wp):
    """Per-core input dicts (shared by kernel() and test.py's bench)."""
    in_maps = []
    for c in range(NCORES):
        bidx, g = divmod(c, HG)
        cs = slice(g * CSL, (g + 1) * CSL)
        maskT = attention_mask[bidx, 0].T.astype(np.float16)  # [M, N]
        maskd = np.ascontiguousarray(
            np.repeat(maskT[:, None, :], 2, axis=1)
        )  # [M, 2, N]
        in_maps.append(
            {
                "xqT": np.ascontiguousarray(inputs_q[bidx].T.astype(np.float16)),
                "xkvT": np.ascontiguousarray(inputs_kv[bidx].T.astype(np.float16)),
                "maskdT": maskd,
                "wq": np.ascontiguousarray(Wq[:, cs].astype(np.float16)),
                "wk": np.ascontiguousarray(Wk[:, cs].astype(np.float16)),
                "wv": np.ascontiguousarray(Wv[:, cs].astype(np.float16)),
                "wp": np.ascontiguousarray(Wp[cs, :].astype(np.float16)),
                "bq2": np.ascontiguousarray(
                    bq[cs].astype(np.float32).reshape(2, 128).T
                ),
                "bk2": np.ascontiguousarray(
                    bk[cs].astype(np.float32).reshape(2, 128).T
                ),
                "bv1": np.ascontiguousarray(bv[cs].reshape(1, CSL).astype(np.float16)),
                "sel4in": _SEL4,
            }
        )
    return in_maps


def kernel(
    inputs_kv, inputs_q, attention_mask, Wq, bq, Wk, bk, Wv, bv, Wp, bp, **_unused
):
    inputs_kv = np.asarray(inputs_kv, dtype=np.float32)
    inputs_q = np.asarray(inputs_q, dtype=np.float32)
    attention_mask = np.asarray(attention_mask)
    bp = np.asarray(bp, dtype=np.float32)

    in_maps = make_in_maps(
        inputs_q, inputs_kv, attention_mask,
        np.asarray(Wq, dtype=np.float32), np.asarray(bq, dtype=np.float32),
        np.asarray(Wk, dtype=np.float32), np.asarray(bk, dtype=np.float32),
        np.asarray(Wv, dtype=np.float32), np.asarray(bv, dtype=np.float32),
        np.asarray(Wp, dtype=np.float32),
    )

    nc = _get_nc()
    res = run_bass_kernel_spmd(
        nc, in_maps, core_ids=list(range(NCORES)), trace=_TRACE
    )
    global _LAST_EXEC_NS
    _LAST_EXEC_NS = res.exec_time_ns

    out = np.zeros((B, N, D), dtype=np.float32)
    for c in range(NCORES):
        bidx = c // HG
        out[bidx] += res.results[c]["outp"].astype(np.float32)
    out += bp
    return out
